# revision 1
# baseline (speedup 1.0000x reference)
"""Trainium2 Bass kernel for nn_EuclideanGATLayer (GAT layer, N=8192) — v3.

Math (per reference):
    Wh = h @ W; s = Wh@a[:F]; d = Wh@a[F:]
    e_ij = leaky_relu(s_i + d_j, 0.01); attn = softmax(mask(e)); out = elu(attn @ Wh)

Key identities:
    exp(lrelu(x)) = exp(0.01 s_i) * Q_j * exp(0.99 relu(s_i+d_j)),  Q_j = exp(0.01 d_j)
    exp(0.99 relu(x)) = max(E_i * D_j, 1),  E_i = exp(0.99 s_i), D_j = exp(0.99 d_j)
    (row factor exp(0.01 s_i) cancels in softmax; Q folded into G = [Q*Wh | Q])

So the stream work is Z_ij = max(E_i*D_j, 1) * A_ij with A in {0,1} staged fp8 —
no transcendental over N^2; values (not logits) tolerate bf16.

Sorting (host staging only; results order-invariant): columns j sorted by d,
rows i sorted by s and striped across cores (so one shared SPMD program fits
all cores). "slab x chunk" blocks with s_max+d_max < -0.02 have relu == 0
exactly => Z = A: those regions feed the matmul DIRECTLY from the fp8 mask
(zero elementwise work). Boundaries come from exact fp64 host s/d; the device
recomputes s/d to ~1e-4 so the classification is provably consistent.

Logit exactness: h and W@a vectors staged/split as bf16 + bf16-residual; d,s
assembled from bf16 matmuls (abs err ~1e-4); E/D/Q from fp32 exps on device.

Per-chunk GEN-region lanes (load-balanced across engines at build time):
    LB: DVE ts 4x t=max(E*D,1); DVE tt t*A(fp8) 1x
    LE: Act widen A->bf16; DVE ts 4x; DVE tt 2x t*Aw
    LA: Act Exp u=E*D (from logits); DVE stt (u max 1)*A 1x
    LG: Act Exp u; DVE ts 4x t=max(u,1); Pool tt t*A(fp8)
    LP: Pool widen A->bf16; DVE ts 4x; DVE tt 2x
"""
import sys

sys.path.insert(0, "/opt/trn_rl_repo")

import numpy as np
from contextlib import ExitStack

import ml_dtypes
import concourse.bass as bass
import concourse.bacc as bacc
import concourse.tile as tile
from concourse import mybir
from concourse import bass_utils
from concourse.masks import make_identity

N_FULL = 8192
IN_DIM = 128
F = 64
NCORES = 8
ROWS = N_FULL // NCORES      # 1024 rows per core
NCH = N_FULL // 128          # 64 j-chunks
NSLAB = 8                    # i-slabs per core (128 sorted rows each)
MARGIN = 0.02

f32 = mybir.dt.float32
f32r = mybir.dt.float32r
bf16 = mybir.dt.bfloat16
fp8 = mybir.dt.float8e4
AF = mybir.ActivationFunctionType
OP = mybir.AluOpType

bf16_np = ml_dtypes.bfloat16
fp8_np = ml_dtypes.float8_e4m3


def plan_lanes(kneg, kpos):
    """Per-chunk lanes: early chunks on DVE (PE is DMA-paced there anyway),
    late chunks on Act/Pool so the post-DMA phase isn't DVE-bound."""
    lanes = []
    alt = 0
    for c in range(NCH):
        if kpos[c] == kneg[c]:
            lanes.append(None)
            continue
        if c < 34:
            lanes.append("LB")
        else:
            lanes.append("LA")
            alt += 1
    return lanes, np.zeros(3)


def gat_body(tc, out, A, WT, hTb, hTr, hTbown, hTrown, W, a, kneg, kpos, lanes, repeats=1):
    nc = tc.nc
    GRP = 8
    ngroups = NCH // GRP

    with ExitStack() as ctx:
        const = ctx.enter_context(tc.tile_pool(name="const", bufs=1))

        # ---- small loads (WT/a first: Wa = WT^T @ a on PE gates everything) ----
        WTsb = const.tile([F, IN_DIM], f32)
        nc.sync.dma_start(out=WTsb, in_=WT)
        a12 = const.tile([F, 2], f32)
        nc.sync.dma_start(
            out=a12, in_=bass.AP(tensor=a.tensor, offset=a.offset,
                                 ap=[[1, F], [F, 2]])
        )
        Wsb = const.tile([128, F], f32)
        nc.sync.dma_start(out=Wsb, in_=W)
        hTbownsb = const.tile([128, ROWS], bf16)
        nc.sync.dma_start(out=hTbownsb, in_=hTbown)
        hTrownsb = const.tile([128, ROWS], fp8)
        nc.sync.dma_start(out=hTrownsb, in_=hTrown)

        Wa1 = Wa2 = None

        def split_bf16(x_f32, name):
            hi = const.tile([128, 1], bf16, name=f"{name}_hi")
            nc.vector.tensor_copy(hi, x_f32)
            hif = const.tile([128, 1], f32, name=f"{name}_hif")
            nc.vector.tensor_copy(hif, hi)
            res = const.tile([128, 1], f32, name=f"{name}_resf")
            nc.vector.tensor_tensor(out=res, in0=x_f32, in1=hif, op=OP.subtract)
            lo = const.tile([128, 1], bf16, name=f"{name}_lo")
            nc.vector.tensor_copy(lo, res)
            return hi, lo

        W2b = const.tile([128, F + 2], bf16)
        s_stat = const.tile([128, 2], bf16)   # [wb1 | wr1]

        def _do_splits():
            wb1, wr1 = split_bf16(Wa1, "wa1")
            wb2, wr2 = split_bf16(Wa2, "wa2")
            nc.vector.tensor_copy(W2b[:, :F], Wsb)
            nc.vector.tensor_copy(W2b[:, F : F + 1], wb2)
            nc.vector.tensor_copy(W2b[:, F + 1 : F + 2], wr2)
            nc.vector.tensor_copy(s_stat[:, 0:1], wb1)
            nc.vector.tensor_copy(s_stat[:, 1:2], wr1)

        ident = const.tile([128, 128], f32)
        make_identity(nc, ident)
        ones2f = const.tile([2, 128], f32)
        nc.vector.memset(ones2f, 1.0)
        ones2r = const.tile([2, 128], f32r)
        nc.vector.tensor_copy(ones2r, ones2f)
        ones1r = const.tile([1, 128], f32r)
        nc.vector.tensor_copy(ones1r, ones2f[0:1, :])

        s_bcast = const.tile([128, ROWS], f32)
        E_bcast = const.tile([128, ROWS], bf16)
        E_col = const.tile([128, NSLAB], f32)
        Gall = const.tile([128, NCH, F + 1], bf16)
        Gpos = const.tile([128, NCH, F + 1], bf16)
        znull = const.tile([128, 512], bf16)
        nc.vector.memset(znull, 0.0)
        dall = const.tile([128, NCH], f32)
        d99 = const.tile([128, NCH], f32)
        Dall = const.tile([128, NCH], f32)
        Qall = const.tile([128, NCH], f32)
        DQall = const.tile([128, NCH], f32)

        hTbg = [const.tile([128, GRP * 128], bf16, name=f"hTbg{g}") for g in range(ngroups)]
        hTrg = [const.tile([128, GRP * 128], fp8, name=f"hTrg{g}") for g in range(ngroups)]
        MB = 4  # mask chunks per batched DMA
        Agrp = [const.tile([128, MB, ROWS], fp8, name=f"A{b}") for b in range(NCH // MB)]

        def mask_chunk(c):
            return Agrp[c // MB][:, c % MB, :]

        def issue_hT_dmas(g):
            nc.sync.dma_start(out=hTbg[g], in_=hTb[:, g * GRP * 128:(g + 1) * GRP * 128])
            nc.sync.dma_start(out=hTrg[g], in_=hTr[:, g * GRP * 128:(g + 1) * GRP * 128])

        def issue_mask_dmas(rep):
            # hT groups run one ahead of mask batches (rep 0 only)
            if rep == 0:
                issue_hT_dmas(0)
                issue_hT_dmas(1)
            for b in range(NCH // MB):
                g = b // (GRP // MB) + 2
                if rep == 0 and b % (GRP // MB) == 0 and g < ngroups:
                    issue_hT_dmas(g)
                nc.sync.dma_start(
                    out=Agrp[b],
                    in_=A[b * MB * 128:(b + 1) * MB * 128, :].rearrange(
                        "(c p) i -> p c i", p=128),
                )

        otpool = ctx.enter_context(tc.tile_pool(name="ot_ps", bufs=1, space="PSUM"))
        ot = [otpool.tile([F + 1, 512], f32, tag=f"ot{t}", name=f"ot{t}") for t in range(2)]

        with ExitStack() as sctx:
            spool = sctx.enter_context(tc.tile_pool(name="setup_s_ps", bufs=2, space="PSUM"))
            wa_ps = spool.tile([128, 2], f32, tag="wa", bufs=1)
            nc.tensor.matmul(wa_ps, WTsb, a12, start=True, stop=True)
            Wa1 = const.tile([128, 1], f32)
            nc.vector.tensor_copy(Wa1, wa_ps[:, 0:1])
            Wa2 = const.tile([128, 1], f32)
            nc.vector.tensor_copy(Wa2, wa_ps[:, 1:2])
            _do_splits()
            # s pieces: a: rows 0-1 = [wb1|wr1]^T b_own; b: row 0 = wb1^T r_own
            for j0 in range(0, ROWS, 512):
                s_ps_a = spool.tile([2, 512], f32, tag="sa", bufs=1)
                s_ps_b = spool.tile([1, 512], f32, tag="sbp", bufs=1)
                nc.tensor.matmul(s_ps_a, s_stat, hTbownsb[:, j0:j0 + 512],
                                 start=True, stop=True)
                nc.tensor.matmul(s_ps_b, s_stat[:, 0:1], hTrownsb[:, j0:j0 + 512],
                                 start=True, stop=True)
                s_sb_a = const.tile([2, 512], f32r, name=f"ssba{j0}")
                nc.vector.tensor_copy(s_sb_a, s_ps_a)
                s_sb_b = const.tile([1, 512], f32r, name=f"ssbb{j0}")
                nc.vector.tensor_copy(s_sb_b, s_ps_b)
                sb_ps = spool.tile([128, 512], f32, tag="sb", bufs=1)
                nc.tensor.matmul(sb_ps, ones2r, s_sb_a, start=True, stop=False)
                nc.tensor.matmul(sb_ps, ones1r, s_sb_b, start=False, stop=True)
                nc.vector.tensor_copy(s_bcast[:, j0:j0 + 512], sb_ps)
            nc.scalar.activation(E_bcast, s_bcast, AF.Exp, scale=0.99)

        issue_mask_dmas(0)

        # ---- main stream (Wh/G setup for group g woven in before chunks 8g..) ----
        whpool = ctx.enter_context(tc.tile_pool(name="setup_wh_ps", bufs=2, space="PSUM"))
        zpool = ctx.enter_context(tc.tile_pool(name="zpool", bufs=8))
        tpool = ctx.enter_context(tc.tile_pool(name="tpool", bufs=8))
        upool = ctx.enter_context(tc.tile_pool(name="upool", bufs=4))

        def setup_scol():
            # s per own row, on row-partitions: PSUM-accumulate all 4 pieces
            wh_s = whpool.tile([128, GRP, 128], f32, tag="wh")
            for b in range(NSLAB):
                own = hTbownsb[:, b * 128:(b + 1) * 128]
                ownr = hTrownsb[:, b * 128:(b + 1) * 128]
                nc.tensor.matmul(wh_s[:, b, 0:2], own, s_stat,
                                 start=True, stop=False)
                nc.tensor.matmul(wh_s[:, b, 0:2], ownr, s_stat,
                                 start=False, stop=True)
            scsum = const.tile([128, NSLAB], f32, name="scsum")
            nc.vector.tensor_reduce(out=scsum, in_=wh_s[:, :, 0:2],
                                    axis=mybir.AxisListType.X, op=OP.add)
            nc.scalar.activation(E_col, scsum, AF.Exp, scale=0.99)

        def setup_group(g):
            wh_grp = whpool.tile([128, GRP, 128], f32, tag="wh")
            for cc in range(GRP):
                j0 = cc * 128
                nc.tensor.matmul(wh_grp[:, cc, :F], hTbg[g][:, j0:j0 + 128],
                                 W2b[:, :F], start=True, stop=True)
                nc.tensor.matmul(wh_grp[:, cc, F:F + 2], hTbg[g][:, j0:j0 + 128],
                                 W2b[:, F:F + 2], start=True, stop=False)
                nc.tensor.matmul(wh_grp[:, cc, F:F + 2], hTrg[g][:, j0:j0 + 128],
                                 W2b[:, F:F + 2], start=False, stop=True)
            sl = slice(g * GRP, (g + 1) * GRP)
            nc.vector.tensor_reduce(out=dall[:, sl], in_=wh_grp[:, :, F:F + 2],
                                    axis=mybir.AxisListType.X, op=OP.add)
            nc.vector.tensor_scalar(out=d99[:, sl], in0=dall[:, sl], scalar1=0.99,
                                    scalar2=None, op0=OP.mult)
            nc.scalar.activation(Dall[:, sl], d99[:, sl], AF.Exp)
            nc.scalar.activation(Qall[:, sl], dall[:, sl], AF.Exp, scale=0.01)
            qb = bass.AP(tensor=Qall.tensor, offset=Qall.offset + g * GRP,
                         ap=[Qall.ap[0], [1, GRP], [0, F]])
            nc.vector.tensor_tensor(out=Gall[:, sl, :F], in0=wh_grp[:, :, :F],
                                    in1=qb, op=OP.mult)
            nc.vector.tensor_copy(Gall[:, sl, F], Qall[:, sl])
            # Gpos = D (bcast) * Gall: num cols D*Q*Wh, den col D*Q = exp(d)
            db_ = bass.AP(tensor=Dall.tensor, offset=Dall.offset + g * GRP,
                          ap=[Dall.ap[0], [1, GRP], [0, F + 1]])
            nc.gpsimd.tensor_tensor(out=Gpos[:, sl, :], in0=Gall[:, sl, :],
                                    in1=db_, op=OP.mult)

        otp = [otpool.tile([F + 1, 512], f32, tag=f"otp{t}", name=f"otp{t}")
               for t in range(2)]

        for rep in range(repeats):
            if rep > 0:
                issue_mask_dmas(rep)
            for c in range(NCH):
                if rep == 0 and c == 16:
                    setup_scol()
                if rep == 0 and c == 0:
                    setup_group(0)
                if rep == 0 and c % GRP == 4 and c // GRP + 1 < ngroups:
                    setup_group(c // GRP + 1)
                if rep == 0 and c == 0:
                    # otp zero-init via dummies; ot is fully covered by chunk 0's
                    # real NEG+band matmuls (host_prep guarantees kpos[0]==NSLAB)
                    for t in range(2):
                        nc.tensor.matmul(otp[t], Gall[:, 0, :], znull, start=True, stop=False)
                k = kneg[c]
                p = kpos[c]
                lane = lanes[c]
                r0 = 128 * k
                r1 = 128 * p
                Ac = mask_chunk(c)
                Z = None
                if lane is not None:
                    sl_b = slice(r0, r1)
                    if lane in ("LB", "LE", "LP"):
                        t_t = tpool.tile([128, ROWS], bf16, tag="t")
                        nc.vector.tensor_scalar(out=t_t[:, sl_b], in0=E_bcast[:, sl_b],
                                                scalar1=Dall[:, c:c + 1], scalar2=1.0,
                                                op0=OP.mult, op1=OP.max)
                        Z = zpool.tile([128, ROWS], bf16, tag="z")
                        if lane == "LB":
                            nc.vector.tensor_tensor(out=Z[:, sl_b], in0=t_t[:, sl_b],
                                                    in1=Ac[:, sl_b], op=OP.mult)
                        else:
                            Aw = upool.tile([128, ROWS], bf16, tag="aw")
                            if lane == "LE":
                                nc.scalar.activation(Aw[:, sl_b], Ac[:, sl_b], AF.Copy)
                            else:
                                nc.gpsimd.tensor_copy(Aw[:, sl_b], Ac[:, sl_b])
                            nc.vector.tensor_tensor(out=Z[:, sl_b], in0=t_t[:, sl_b],
                                                    in1=Aw[:, sl_b], op=OP.mult)
                    else:  # LA / LG
                        u = upool.tile([128, ROWS], bf16, tag="u")
                        nc.scalar.activation(u[:, sl_b], s_bcast[:, sl_b], AF.Exp,
                                             bias=d99[:, c:c + 1], scale=0.99)
                        Z = zpool.tile([128, ROWS], bf16, tag="z")
                        if lane == "LA":
                            nc.vector.scalar_tensor_tensor(out=Z[:, sl_b], in0=u[:, sl_b],
                                                           scalar=1.0, in1=Ac[:, sl_b],
                                                           op0=OP.max, op1=OP.mult)
                        else:
                            t_t = tpool.tile([128, ROWS], bf16, tag="t")
                            nc.vector.tensor_scalar(out=t_t[:, sl_b], in0=u[:, sl_b],
                                                    scalar1=1.0, scalar2=None, op0=OP.max)
                            nc.gpsimd.tensor_tensor(out=Z[:, sl_b], in0=t_t[:, sl_b],
                                                    in1=Ac[:, sl_b], op=OP.mult)

                last = rep == repeats - 1 and c == NCH - 1
                G_c = Gall[:, c, :]
                Gp_c = Gpos[:, c, :]
                for half in range(2):
                    lo, hi = 512 * half, 512 * (half + 1)
                    b0 = min(max(r0, lo), hi)   # NEG | band boundary
                    b1 = min(max(r1, lo), hi)   # band | POS boundary
                    st = rep == 0 and c == 0
                    if b0 > lo:  # NEG: weight exactly Q_j -> moving = A
                        nc.tensor.matmul(ot[half][:, lo - 512 * half:b0 - 512 * half],
                                         G_c, Ac[:, lo:b0], start=st, stop=False)
                    if b1 > b0:  # band: moving = Z
                        nc.tensor.matmul(ot[half][:, b0 - 512 * half:b1 - 512 * half],
                                         G_c, Z[:, b0:b1], start=st, stop=False)
                    if hi > b1:  # POS: weight E_i*D_j*Q_j -> moving = A vs Gpos
                        nc.tensor.matmul(otp[half][:, b1 - 512 * half:],
                                         Gp_c, Ac[:, b1:hi], start=False, stop=False)
                if last:
                    for t in range(2):
                        nc.tensor.matmul(ot[t][:, :16], Gall[:, 0, :], znull[:, :16],
                                         start=False, stop=True, skip_group_check=True)
                        nc.tensor.matmul(otp[t][:, :16], Gall[:, 0, :], znull[:, :16],
                                         start=False, stop=True, skip_group_check=True)

        # ---- epilogue ----
        small = ctx.enter_context(tc.tile_pool(name="small", bufs=2))
        otsb, otpsb, tpws, hpes, hptots, denss = {}, {}, {}, {}, {}, {}
        for half in range(2):
            otsb[half] = small.tile([F + 1, 512], f32, tag="otsb", name=f"otsb{half}")
            nc.vector.tensor_copy(otsb[half], ot[half])
            otpsb[half] = small.tile([F + 1, 512], f32, tag="otpsb", name=f"otpsb{half}")
            nc.scalar.activation(otpsb[half], otp[half], AF.Copy)
        for half in range(2):
            tpw = whpool.tile([128, GRP, 128], f32, tag="wh")
            tpws[half] = tpw
            for q in range(4):
                nc.tensor.transpose(tpw[:, q, :F + 1],
                                    otsb[half][:, q * 128:(q + 1) * 128],
                                    ident[:F + 1, :F + 1])
                nc.tensor.transpose(tpw[:, 4 + q, :F + 1],
                                    otpsb[half][:, q * 128:(q + 1) * 128],
                                    ident[:F + 1, :F + 1])
        for half in range(2):
            tp4 = tpws[half][:, :4, :F + 1]
            tp4p = tpws[half][:, 4:, :F + 1]
            ecb = bass.AP(tensor=E_col.tensor, offset=E_col.offset + half * 4,
                          ap=[E_col.ap[0], [1, 4], [0, F + 1]])
            hpes[half] = small.tile([128, 4, F + 1], f32, tag="hpe", name=f"hpe{half}")
            nc.vector.tensor_tensor(out=hpes[half], in0=tp4p, in1=ecb, op=OP.mult)
            hptots[half] = small.tile([128, 4, F + 1], f32, tag="hptot", name=f"hptot{half}")
            nc.vector.tensor_tensor(out=hptots[half], in0=tp4, in1=hpes[half], op=OP.add)
            denss[half] = small.tile([128, 4], f32, tag="dens", name=f"dens{half}")
            nc.vector.reciprocal(denss[half], hptots[half][:, :, F])
        for half in range(2):
            hptot, dens = hptots[half], denss[half]
            db = bass.AP(tensor=dens.tensor, offset=dens.offset,
                         ap=[dens.ap[0], dens.ap[1], [0, F]])
            hpre = small.tile([128, 4, F], f32, tag="hpre")
            nc.vector.tensor_tensor(out=hpre, in0=hptot[:, :, :F], in1=db, op=OP.mult)
            # elu(x) = relu(x) - 1 + exp(min(x, 0))
            emin = small.tile([128, 4, F], f32, tag="emin")
            nc.vector.tensor_scalar(out=emin, in0=hpre, scalar1=0.0, scalar2=None,
                                    op0=OP.min)
            eexp = small.tile([128, 4, F], f32, tag="eexp")
            nc.scalar.activation(eexp, emin, AF.Exp)
            relu1 = small.tile([128, 4, F], f32, tag="relu1")
            nc.gpsimd.tensor_scalar(out=relu1, in0=hpre, scalar1=0.0, scalar2=-1.0,
                                    op0=OP.max, op1=OP.add)
            otf = small.tile([128, 4, F], bf16, tag="otf")
            nc.vector.tensor_tensor(out=otf, in0=relu1, in1=eexp, op=OP.add)
            nc.sync.dma_start(
                out=out[512 * half:512 * (half + 1), :].rearrange(
                    "(b p) f -> p b f", p=128),
                in_=otf)


def build_module(kneg, kpos, lanes, repeats=1):
    nc = bacc.Bacc("TRN2", target_bir_lowering=False, debug=False,
                   enable_asserts=True, num_devices=NCORES)
    A = nc.dram_tensor("A", [N_FULL, ROWS], fp8, kind="ExternalInput").ap()
    WT = nc.dram_tensor("WT", [F, IN_DIM], f32, kind="ExternalInput").ap()
    hTb = nc.dram_tensor("hTb", [IN_DIM, N_FULL], bf16, kind="ExternalInput").ap()
    hTr = nc.dram_tensor("hTr", [IN_DIM, N_FULL], fp8, kind="ExternalInput").ap()
    hTbown = nc.dram_tensor("hTbown", [IN_DIM, ROWS], bf16, kind="ExternalInput").ap()
    hTrown = nc.dram_tensor("hTrown", [IN_DIM, ROWS], fp8, kind="ExternalInput").ap()
    W = nc.dram_tensor("W", [IN_DIM, F], f32, kind="ExternalInput").ap()
    a = nc.dram_tensor("a", [2 * F], f32, kind="ExternalInput").ap()
    out = nc.dram_tensor("out", [ROWS, F], bf16, kind="ExternalOutput").ap()
    with tile.TileContext(nc) as tc:
        gat_body(tc, out, A, WT, hTb, hTr, hTbown, hTrown, W, a, kneg, kpos, lanes,
                 repeats=repeats)
    nc.compile()
    return nc


def host_prep(h, adj, W, a):
    h64 = np.asarray(h, dtype=np.float64)
    W64 = np.asarray(W, dtype=np.float64)
    a64 = np.asarray(a, dtype=np.float64)
    Wh = h64 @ W64
    s_full = Wh @ a64[:F]
    d_full = Wh @ a64[F:]
    sigma = np.argsort(d_full, kind="stable")
    rho = np.argsort(s_full, kind="stable")
    s_sorted = s_full[rho]
    d_sorted = d_full[sigma]

    kneg, kpos = [], []
    for c in range(NCH):
        dmax = d_sorted[128 * (c + 1) - 1]
        dmin = d_sorted[128 * c]
        k = 0
        while k < NSLAB and s_sorted[1024 * (k + 1) - 1] + dmax < -MARGIN:
            k += 1
        kneg.append(k)
        p = NSLAB
        while p > k and s_sorted[1024 * (p - 1)] + dmin > MARGIN:
            p -= 1
        kpos.append(p)

    hf = np.asarray(h, dtype=np.float32)
    hb = hf.astype(bf16_np)
    hr = (hf - hb.astype(np.float32)).astype(fp8_np)
    hTbs = np.ascontiguousarray(hb.T[:, sigma])
    hTrs = np.ascontiguousarray(hr.T[:, sigma])

    rows = [rho[c::NCORES] for c in range(NCORES)]
    adjs = np.asarray(adj)
    in_maps = []
    for c in range(NCORES):
        rc = rows[c]
        Ac = np.ascontiguousarray(adjs[rc][:, sigma].T).astype(fp8_np)
        in_maps.append({
            "A": Ac,
            "WT": np.ascontiguousarray(np.asarray(W, dtype=np.float32).T),
            "hTb": hTbs,
            "hTr": hTrs,
            "hTbown": np.ascontiguousarray(hb.T[:, rc]),
            "hTrown": np.ascontiguousarray(hr.T[:, rc]),
            "W": np.asarray(W, dtype=np.float32),
            "a": np.asarray(a, dtype=np.float32),
        })
    return kneg, kpos, rows, in_maps


_nc_cache = {}


def get_module(kneg, kpos, repeats=1):
    key = (tuple(kneg), tuple(kpos), repeats)
    if key not in _nc_cache:
        lanes, tot = plan_lanes(kneg, kpos)
        print(f"lane plan: engine busy estimate (DVE/Act/Pool) = {np.round(tot, 1)}")
        _nc_cache[key] = build_module(kneg, kpos, lanes, repeats=repeats)
    return _nc_cache[key]


def kernel(h, adj, W, a, trace=False, trace_kwargs=None, repeats=1):
    kneg, kpos, rows, in_maps = host_prep(h, adj, W, a)
    nc = get_module(kneg, kpos, repeats)
    res = bass_utils.run_bass_kernel_spmd(
        nc, in_maps, core_ids=list(range(NCORES)), trace=trace,
        **(trace_kwargs or {}))
    out = np.empty((N_FULL, F), dtype=np.float32)
    for c in range(NCORES):
        out[rows[c]] = np.asarray(res.results[c]["out"]).astype(np.float32)
    kernel.last_results = res
    return out


if __name__ == "__main__":
    rng = np.random.default_rng(0)
    h = rng.standard_normal((N_FULL, IN_DIM), dtype=np.float32)
    adj = (rng.random((N_FULL, N_FULL)) < 0.5).astype(np.int32)
    W = (rng.standard_normal((IN_DIM, F), dtype=np.float32) / np.sqrt(IN_DIM))
    a = rng.standard_normal(2 * F, dtype=np.float32)
    out = kernel(h, adj, W, a)
    print("out", out.shape, np.abs(out).mean())



# revision 5
# speedup vs baseline: 1.0547x; 1.0547x over previous
"""Trainium2 Bass kernel for nn_EuclideanGATLayer (GAT layer, N=8192) — v4.

Math (per reference):
    Wh = h @ W; s = Wh@a[:F]; d = Wh@a[F:]
    e_ij = leaky_relu(s_i + d_j, 0.01); attn = softmax(mask(e)); out = elu(attn @ Wh)

Key identities (unchanged from v3):
    exp(lrelu(x)) = exp(0.01 s_i) * Q_j * exp(0.99 relu(s_i+d_j)),  Q_j = exp(0.01 d_j)
    exp(0.99 relu(x)) = max(E_i * D_j, 1),  E_i = exp(0.99 s_i), D_j = exp(0.99 d_j)
    (row factor exp(0.01 s_i) cancels in softmax; Q folded into G = [Q*Wh | Q])

Sorting (host staging only; results order-invariant): columns j sorted by d,
rows i sorted by s and striped across cores. "slab x chunk" blocks with
s_max+d_max < -MARGIN have relu == 0 exactly => stream weight = Q_j: those NEG
regions feed the matmul directly from the fp8 mask. POS blocks (s_min+d_min >
MARGIN) use weight E_i*D_j*Q_j with E_i folded into the epilogue (otp psum).
Band blocks compute Z = max(E*D,1)*A elementwise (bf16, DVE/Act/Pool lanes).

v4 news:
  * NEG regions are processed per chunk-PAIR with one fp8 DoubleRow matmul
    (stationary = fp8(G) pairs padded to 80 cols so the pair stride is
    16-byte aligned; moving = the two chunks' mask slices). NEG weights are
    diffuse (Q in [0.73,1.36]) so single fp8 stationary is accurate enough.
  * d/s low bits: instead of fp8 residual copies of h (1.1MB DMA), the host
    stages exact correction vectors dlo/slo/sloc = (fp64 logit) - (the bf16
    path the device computes). Device adds them before the exps.
  * masks arrive in 8 x 1MB DMAs interleaved with the hT group loads so the
    (serialized) DMA track stays solid; all of A stays resident in SBUF.

Logit exactness: d_dev = bf16-matmul part + dlo => |d_dev - d_fp64| ~ 1e-6.
"""
import sys

sys.path.insert(0, "/opt/trn_rl_repo")

import numpy as np
from contextlib import ExitStack

import ml_dtypes
import concourse.bass as bass
import concourse.bacc as bacc
import concourse.tile as tile
from concourse import mybir
from concourse import bass_utils
from concourse.masks import make_identity

N_FULL = 8192
IN_DIM = 128
F = 64
NCORES = 8
ROWS = N_FULL // NCORES      # 1024 rows per core
NCH = N_FULL // 128          # 64 j-chunks
NPAIR = NCH // 2
NSLAB = 8                    # i-slabs per core (128 sorted rows each)
MARGIN = 0.02
GP = 80                      # padded fp8-G width (pair stride must be %16)
MB = 8                       # mask chunks per batched DMA

f32 = mybir.dt.float32
f32r = mybir.dt.float32r
bf16 = mybir.dt.bfloat16
fp8 = mybir.dt.float8e4
AF = mybir.ActivationFunctionType
OP = mybir.AluOpType
DR = mybir.MatmulPerfMode.DoubleRow

bf16_np = ml_dtypes.bfloat16
fp8_np = ml_dtypes.float8_e4m3


def plan_lanes(kneg, kpos):
    """Per-chunk band lanes: early chunks on DVE (PE is DMA-paced there),
    late chunks on Act/Pool so the post-DMA phase isn't DVE-bound."""
    lanes = []
    for c in range(NCH):
        if kpos[c] == kneg[c]:
            lanes.append(None)
        elif c < 34:
            lanes.append("LB")
        else:
            lanes.append("LA")
    return lanes


def gat_body(tc, out, A, WT, hTb, hTbown, W, a, dlo, slo, sloc, kneg, kpos, lanes):
    nc = tc.nc
    GRP = 8
    ngroups = NCH // GRP

    with ExitStack() as ctx:
        const = ctx.enter_context(tc.tile_pool(name="const", bufs=1))

        # ---- small loads (WT/a first: Wa = WT^T @ a on PE gates everything) ----
        WTsb = const.tile([F, IN_DIM], f32)
        nc.sync.dma_start(out=WTsb, in_=WT)
        a12 = const.tile([F, 2], f32)
        nc.sync.dma_start(
            out=a12, in_=bass.AP(tensor=a.tensor, offset=a.offset,
                                 ap=[[1, F], [F, 2]])
        )
        Wsb = const.tile([128, F], f32)
        nc.sync.dma_start(out=Wsb, in_=W)
        dlosb = const.tile([128, NCH], f32)
        nc.sync.dma_start(out=dlosb, in_=dlo)
        slosb = const.tile([1, ROWS], f32)
        nc.sync.dma_start(out=slosb, in_=slo)
        slocsb = const.tile([128, NSLAB], f32)
        nc.sync.dma_start(out=slocsb, in_=sloc)
        hTbownsb = const.tile([128, ROWS], bf16)
        nc.sync.dma_start(out=hTbownsb, in_=hTbown)

        Wa1 = Wa2 = None

        def split_bf16(x_f32, name):
            hi = const.tile([128, 1], bf16, name=f"{name}_hi")
            nc.vector.tensor_copy(hi, x_f32)
            hif = const.tile([128, 1], f32, name=f"{name}_hif")
            nc.vector.tensor_copy(hif, hi)
            res = const.tile([128, 1], f32, name=f"{name}_resf")
            nc.vector.tensor_tensor(out=res, in0=x_f32, in1=hif, op=OP.subtract)
            lo = const.tile([128, 1], bf16, name=f"{name}_lo")
            nc.vector.tensor_copy(lo, res)
            return hi, lo

        W2b = const.tile([128, F + 2], bf16)
        s_stat = const.tile([128, 2], bf16)   # [wb1 | wr1]

        def _do_splits():
            wb1, wr1 = split_bf16(Wa1, "wa1")
            wb2, wr2 = split_bf16(Wa2, "wa2")
            nc.vector.tensor_copy(W2b[:, :F], Wsb)
            nc.vector.tensor_copy(W2b[:, F : F + 1], wb2)
            nc.vector.tensor_copy(W2b[:, F + 1 : F + 2], wr2)
            nc.vector.tensor_copy(s_stat[:, 0:1], wb1)
            nc.vector.tensor_copy(s_stat[:, 1:2], wr1)

        ident = const.tile([128, 128], f32)
        make_identity(nc, ident)
        ones2f = const.tile([2, 128], f32)
        nc.vector.memset(ones2f, 1.0)
        ones2r = const.tile([2, 128], f32r)
        nc.vector.tensor_copy(ones2r, ones2f)
        ones1r = const.tile([1, 128], f32r)
        nc.vector.tensor_copy(ones1r, ones2f[0:1, :])

        s_bcast = const.tile([128, ROWS], f32)
        E_bcast = const.tile([128, ROWS], bf16)
        E_col = const.tile([128, NSLAB], f32)
        Gall = const.tile([128, NCH, F + 1], bf16)
        Gpos = const.tile([128, NCH, F + 1], bf16)
        G8 = const.tile([128, NCH, GP], fp8)
        nc.gpsimd.memset(G8[:, :, F + 1:GP], 0.0)
        znull = const.tile([128, 512], bf16)
        nc.vector.memset(znull, 0.0)
        dall = const.tile([128, NCH], f32)
        d99 = const.tile([128, NCH], f32)
        Dall = const.tile([128, NCH], f32)
        Qall = const.tile([128, NCH], f32)

        hTbg = [const.tile([128, GRP * 128], bf16, name=f"hTbg{g}") for g in range(ngroups)]
        Agrp = [const.tile([128, MB, ROWS], fp8, name=f"A{b}") for b in range(NCH // MB)]

        def mask_chunk(c):
            return Agrp[c // MB][:, c % MB, :]

        def issue_stream_dmas():
            # hT group g must land before pair 4g-ish; masks batch b before
            # pair 4b. Interleave so the DMA track never idles.
            nc.sync.dma_start(out=hTbg[0], in_=hTb[:, 0:GRP * 128])
            nc.sync.dma_start(out=hTbg[1], in_=hTb[:, GRP * 128:2 * GRP * 128])
            for b in range(NCH // MB):
                g = b + 2
                nc.sync.dma_start(
                    out=Agrp[b],
                    in_=A[b * MB * 128:(b + 1) * MB * 128, :].rearrange(
                        "(c p) i -> p c i", p=128),
                )
                if g < ngroups:
                    nc.sync.dma_start(
                        out=hTbg[g], in_=hTb[:, g * GRP * 128:(g + 1) * GRP * 128])

        otpool = ctx.enter_context(tc.tile_pool(name="ot_ps", bufs=1, space="PSUM"))
        ot = [otpool.tile([GP, 512], f32, tag=f"ot{t}", name=f"ot{t}") for t in range(2)]

        with ExitStack() as sctx:
            spool = sctx.enter_context(tc.tile_pool(name="setup_s_ps", bufs=2, space="PSUM"))
            wa_ps = spool.tile([128, 2], f32, tag="wa", bufs=1)
            nc.tensor.matmul(wa_ps, WTsb, a12, start=True, stop=True)
            Wa1 = const.tile([128, 1], f32)
            nc.vector.tensor_copy(Wa1, wa_ps[:, 0:1])
            Wa2 = const.tile([128, 1], f32)
            nc.vector.tensor_copy(Wa2, wa_ps[:, 1:2])
            _do_splits()
            # s rows: a: rows 0-1 = [wb1|wr1]^T b_own; slo row appended
            for j0 in range(0, ROWS, 512):
                s_ps_a = spool.tile([2, 512], f32, tag="sa", bufs=1)
                nc.tensor.matmul(s_ps_a, s_stat, hTbownsb[:, j0:j0 + 512],
                                 start=True, stop=True)
                s_sb_a = const.tile([2, 512], f32r, name=f"ssba{j0}")
                nc.vector.tensor_copy(s_sb_a, s_ps_a)
                s_sb_b = const.tile([1, 512], f32r, name=f"ssbb{j0}")
                nc.vector.tensor_copy(s_sb_b, slosb[:, j0:j0 + 512])
                sb_ps = spool.tile([128, 512], f32, tag="sb", bufs=1)
                nc.tensor.matmul(sb_ps, ones2r, s_sb_a, start=True, stop=False)
                nc.tensor.matmul(sb_ps, ones1r, s_sb_b, start=False, stop=True)
                nc.vector.tensor_copy(s_bcast[:, j0:j0 + 512], sb_ps)
            nc.scalar.activation(E_bcast, s_bcast, AF.Exp, scale=0.99)

        issue_stream_dmas()

        # ---- main stream (Wh/G setup for group g woven in before chunks 8g..) ----
        whpool = ctx.enter_context(tc.tile_pool(name="setup_wh_ps", bufs=2, space="PSUM"))
        zpool = ctx.enter_context(tc.tile_pool(name="zpool", bufs=8))
        tpool = ctx.enter_context(tc.tile_pool(name="tpool", bufs=8))
        upool = ctx.enter_context(tc.tile_pool(name="upool", bufs=4))

        def setup_scol():
            # s per own row, on row-partitions
            wh_s = whpool.tile([128, GRP, 128], f32, tag="wh")
            for b in range(NSLAB):
                own = hTbownsb[:, b * 128:(b + 1) * 128]
                nc.tensor.matmul(wh_s[:, b, 0:2], own, s_stat,
                                 start=True, stop=True)
            scsum = const.tile([128, NSLAB], f32, name="scsum")
            nc.vector.tensor_reduce(out=scsum, in_=wh_s[:, :, 0:2],
                                    axis=mybir.AxisListType.X, op=OP.add)
            scs2 = const.tile([128, NSLAB], f32, name="scs2")
            nc.vector.tensor_tensor(out=scs2, in0=scsum, in1=slocsb, op=OP.add)
            nc.scalar.activation(E_col, scs2, AF.Exp, scale=0.99)

        def setup_group(g):
            wh_grp = whpool.tile([128, GRP, 128], f32, tag="wh")
            for cc in range(GRP):
                j0 = cc * 128
                nc.tensor.matmul(wh_grp[:, cc, :F + 2], hTbg[g][:, j0:j0 + 128],
                                 W2b[:, :F + 2], start=True, stop=True)
            sl = slice(g * GRP, (g + 1) * GRP)
            dhi = const.tile([128, GRP], f32, name=f"dhi{g}")
            nc.vector.tensor_reduce(out=dhi, in_=wh_grp[:, :, F:F + 2],
                                    axis=mybir.AxisListType.X, op=OP.add)
            nc.vector.tensor_tensor(out=dall[:, sl], in0=dhi, in1=dlosb[:, sl],
                                    op=OP.add)
            nc.vector.tensor_scalar(out=d99[:, sl], in0=dall[:, sl], scalar1=0.99,
                                    scalar2=None, op0=OP.mult)
            nc.scalar.activation(Dall[:, sl], d99[:, sl], AF.Exp)
            nc.scalar.activation(Qall[:, sl], dall[:, sl], AF.Exp, scale=0.01)
            qb = bass.AP(tensor=Qall.tensor, offset=Qall.offset + g * GRP,
                         ap=[Qall.ap[0], [1, GRP], [0, F]])
            nc.vector.tensor_tensor(out=Gall[:, sl, :F], in0=wh_grp[:, :, :F],
                                    in1=qb, op=OP.mult)
            nc.vector.tensor_copy(Gall[:, sl, F], Qall[:, sl])
            # Gpos = D (bcast) * Gall: num cols D*Q*Wh, den col D*Q = exp(d)
            db_ = bass.AP(tensor=Dall.tensor, offset=Dall.offset + g * GRP,
                          ap=[Dall.ap[0], [1, GRP], [0, F + 1]])
            nc.gpsimd.tensor_tensor(out=Gpos[:, sl, :], in0=Gall[:, sl, :],
                                    in1=db_, op=OP.mult)
            # fp8 copy of G for the DoubleRow NEG matmuls (diffuse weights)
            nc.scalar.activation(G8[:, sl, 0:F + 1], Gall[:, sl, :], AF.Copy)

        otp = [otpool.tile([F + 1, 512], f32, tag=f"otp{t}", name=f"otp{t}")
               for t in range(2)]

        for t in range(NPAIR):
            c0, c1 = 2 * t, 2 * t + 1
            if t == 8:
                setup_scol()
            if t == 0:
                setup_group(0)
            if c0 % GRP == 4 and c0 // GRP + 1 < ngroups:
                setup_group(c0 // GRP + 1)
            if t == 0:
                # otp zero-init via dummies; ot is fully covered by pair 0's
                # real DR+band matmuls (host_prep guarantees kpos[0]==NSLAB)
                for h in range(2):
                    nc.tensor.matmul(otp[h], Gall[:, 0, :], znull, start=True, stop=False)

            rz = 128 * min(kneg[c0], kneg[c1])
            # ---- paired NEG via fp8 DoubleRow: weight Q_j, moving A ----
            if rz > 0:
                b_t = c0 // MB
                sub = c0 % MB
                for h in range(2):
                    lo, hi = 512 * h, 512 * (h + 1)
                    l = max(0, lo)
                    r = min(rz, hi)
                    if r > l:
                        nc.tensor.matmul(
                            ot[h][:, l - lo:r - lo],
                            G8[:, c0:c0 + 2, :],
                            Agrp[b_t][:, sub:sub + 2, l:r],
                            start=(t == 0), stop=False, perf_mode=DR,
                            skip_group_check=True)

            for c in (c0, c1):
                k = kneg[c]
                p = kpos[c]
                lane = lanes[c]
                r0 = 128 * k
                r1 = 128 * p
                Ac = mask_chunk(c)
                Z = None
                if lane is not None:
                    sl_b = slice(r0, r1)
                    if lane == "LB":
                        t_t = tpool.tile([128, ROWS], bf16, tag="t")
                        nc.vector.tensor_scalar(out=t_t[:, sl_b], in0=E_bcast[:, sl_b],
                                                scalar1=Dall[:, c:c + 1], scalar2=1.0,
                                                op0=OP.mult, op1=OP.max)
                        Z = zpool.tile([128, ROWS], bf16, tag="z")
                        nc.vector.tensor_tensor(out=Z[:, sl_b], in0=t_t[:, sl_b],
                                                in1=Ac[:, sl_b], op=OP.mult)
                    else:  # LA
                        u = upool.tile([128, ROWS], bf16, tag="u")
                        nc.scalar.activation(u[:, sl_b], s_bcast[:, sl_b], AF.Exp,
                                             bias=d99[:, c:c + 1], scale=0.99)
                        Z = zpool.tile([128, ROWS], bf16, tag="z")
                        nc.vector.scalar_tensor_tensor(out=Z[:, sl_b], in0=u[:, sl_b],
                                                       scalar=1.0, in1=Ac[:, sl_b],
                                                       op0=OP.max, op1=OP.mult)

                last = c == NCH - 1
                G_c = Gall[:, c, :]
                Gp_c = Gpos[:, c, :]
                for half in range(2):
                    lo, hi = 512 * half, 512 * (half + 1)
                    a0 = min(max(rz, lo), hi)   # pairNEG | ownNEG boundary
                    b0 = min(max(r0, lo), hi)   # NEG | band boundary
                    b1 = min(max(r1, lo), hi)   # band | POS boundary
                    st = c == 0
                    if b0 > a0:  # own-NEG sliver: weight exactly Q_j -> moving = A
                        nc.tensor.matmul(ot[half][:F + 1, a0 - lo:b0 - lo],
                                         G_c, Ac[:, a0:b0], start=st, stop=False,
                                         skip_group_check=True)
                    if b1 > b0:  # band: moving = Z
                        nc.tensor.matmul(ot[half][:F + 1, b0 - lo:b1 - lo],
                                         G_c, Z[:, b0:b1], start=st, stop=False,
                                         skip_group_check=True)
                    if hi > b1:  # POS: weight E_i*D_j*Q_j -> moving = A vs Gpos
                        nc.tensor.matmul(otp[half][:, b1 - lo:],
                                         Gp_c, Ac[:, b1:hi], start=False, stop=False)
                if last:
                    for h in range(2):
                        nc.tensor.matmul(ot[h][:F + 1, :16], Gall[:, 0, :], znull[:, :16],
                                         start=False, stop=True, skip_group_check=True)
                        nc.tensor.matmul(otp[h][:, :16], Gall[:, 0, :], znull[:, :16],
                                         start=False, stop=True, skip_group_check=True)

        # ---- epilogue ----
        small = ctx.enter_context(tc.tile_pool(name="small", bufs=2))
        otsb, otpsb, tpws, hpes, hptots, denss = {}, {}, {}, {}, {}, {}
        for half in range(2):
            otsb[half] = small.tile([F + 1, 512], f32, tag="otsb", name=f"otsb{half}")
            nc.vector.tensor_copy(otsb[half], ot[half][:F + 1, :])
            otpsb[half] = small.tile([F + 1, 512], f32, tag="otpsb", name=f"otpsb{half}")
            nc.scalar.activation(otpsb[half], otp[half], AF.Copy)
        for half in range(2):
            tpw = whpool.tile([128, GRP, 128], f32, tag="wh")
            tpws[half] = tpw
            for q in range(4):
                nc.tensor.transpose(tpw[:, q, :F + 1],
                                    otsb[half][:, q * 128:(q + 1) * 128],
                                    ident[:F + 1, :F + 1])
                nc.tensor.transpose(tpw[:, 4 + q, :F + 1],
                                    otpsb[half][:, q * 128:(q + 1) * 128],
                                    ident[:F + 1, :F + 1])
        for half in range(2):
            tp4 = tpws[half][:, :4, :F + 1]
            tp4p = tpws[half][:, 4:, :F + 1]
            ecb = bass.AP(tensor=E_col.tensor, offset=E_col.offset + half * 4,
                          ap=[E_col.ap[0], [1, 4], [0, F + 1]])
            hpes[half] = small.tile([128, 4, F + 1], f32, tag="hpe", name=f"hpe{half}")
            nc.vector.tensor_tensor(out=hpes[half], in0=tp4p, in1=ecb, op=OP.mult)
            hptots[half] = small.tile([128, 4, F + 1], f32, tag="hptot", name=f"hptot{half}")
            nc.vector.tensor_tensor(out=hptots[half], in0=tp4, in1=hpes[half], op=OP.add)
            denss[half] = small.tile([128, 4], f32, tag="dens", name=f"dens{half}")
            nc.vector.reciprocal(denss[half], hptots[half][:, :, F])
        for half in range(2):
            hptot, dens = hptots[half], denss[half]
            db = bass.AP(tensor=dens.tensor, offset=dens.offset,
                         ap=[dens.ap[0], dens.ap[1], [0, F]])
            hpre = small.tile([128, 4, F], f32, tag="hpre")
            nc.vector.tensor_tensor(out=hpre, in0=hptot[:, :, :F], in1=db, op=OP.mult)
            # elu(x) = relu(x) - 1 + exp(min(x, 0))
            emin = small.tile([128, 4, F], f32, tag="emin")
            nc.vector.tensor_scalar(out=emin, in0=hpre, scalar1=0.0, scalar2=None,
                                    op0=OP.min)
            eexp = small.tile([128, 4, F], f32, tag="eexp")
            nc.scalar.activation(eexp, emin, AF.Exp)
            relu1 = small.tile([128, 4, F], f32, tag="relu1")
            nc.gpsimd.tensor_scalar(out=relu1, in0=hpre, scalar1=0.0, scalar2=-1.0,
                                    op0=OP.max, op1=OP.add)
            otf = small.tile([128, 4, F], bf16, tag="otf")
            nc.vector.tensor_tensor(out=otf, in0=relu1, in1=eexp, op=OP.add)
            nc.sync.dma_start(
                out=out[512 * half:512 * (half + 1), :].rearrange(
                    "(b p) f -> p b f", p=128),
                in_=otf)


def build_module(kneg, kpos, lanes):
    nc = bacc.Bacc("TRN2", target_bir_lowering=False, debug=False,
                   enable_asserts=True, num_devices=NCORES)
    A = nc.dram_tensor("A", [N_FULL, ROWS], fp8, kind="ExternalInput").ap()
    WT = nc.dram_tensor("WT", [F, IN_DIM], f32, kind="ExternalInput").ap()
    hTb = nc.dram_tensor("hTb", [IN_DIM, N_FULL], bf16, kind="ExternalInput").ap()
    hTbown = nc.dram_tensor("hTbown", [IN_DIM, ROWS], bf16, kind="ExternalInput").ap()
    W = nc.dram_tensor("W", [IN_DIM, F], f32, kind="ExternalInput").ap()
    a = nc.dram_tensor("a", [2 * F], f32, kind="ExternalInput").ap()
    dlo = nc.dram_tensor("dlo", [128, NCH], f32, kind="ExternalInput").ap()
    slo = nc.dram_tensor("slo", [1, ROWS], f32, kind="ExternalInput").ap()
    sloc = nc.dram_tensor("sloc", [128, NSLAB], f32, kind="ExternalInput").ap()
    out = nc.dram_tensor("out", [ROWS, F], bf16, kind="ExternalOutput").ap()
    with tile.TileContext(nc) as tc:
        gat_body(tc, out, A, WT, hTb, hTbown, W, a, dlo, slo, sloc,
                 kneg, kpos, lanes)
    nc.compile()
    return nc


def host_prep(h, adj, W, a):
    h64 = np.asarray(h, dtype=np.float64)
    W64 = np.asarray(W, dtype=np.float64)
    a64 = np.asarray(a, dtype=np.float64)
    Wh = h64 @ W64
    s_full = Wh @ a64[:F]
    d_full = Wh @ a64[F:]
    sigma = np.argsort(d_full, kind="stable")
    rho = np.argsort(s_full, kind="stable")
    s_sorted = s_full[rho]
    d_sorted = d_full[sigma]

    kneg, kpos = [], []
    for c in range(NCH):
        dmax = d_sorted[128 * (c + 1) - 1]
        dmin = d_sorted[128 * c]
        k = 0
        while k < NSLAB and s_sorted[1024 * (k + 1) - 1] + dmax < -MARGIN:
            k += 1
        kneg.append(k)
        p = NSLAB
        while p > k and s_sorted[1024 * (p - 1)] + dmin > MARGIN:
            p -= 1
        kpos.append(p)

    hf = np.asarray(h, dtype=np.float32)
    hb = hf.astype(bf16_np)
    hb64 = hb.astype(np.float64)
    hTbs = np.ascontiguousarray(hb.T[:, sigma])

    # replicate the device's Wa splits to compute exact lo-corrections
    Wf = np.asarray(W, dtype=np.float32)
    af = np.asarray(a, dtype=np.float32)
    Wa12 = W64 @ a64.reshape(2, F).T    # [:, 0] = W@a1, [:, 1] = W@a2
    # device computes wa_ps = WT^T @ a12 in f32 PE; model as f64->f32
    Wa1f = Wa12[:, 0].astype(np.float32)
    Wa2f = Wa12[:, 1].astype(np.float32)

    def splits(x):
        hi = x.astype(bf16_np)
        lo = (x - hi.astype(np.float32)).astype(bf16_np)
        return hi.astype(np.float64) + lo.astype(np.float64)

    w1 = splits(Wa1f)   # wb1 + wr1 as the device sees them
    w2 = splits(Wa2f)
    d_hi = hb64 @ w2              # what the device's bf16 path yields (f64 model)
    s_hi = hb64 @ w1
    dlo_full = d_full - d_hi      # exact correction
    slo_full = s_full - s_hi

    dlo_sorted = dlo_full[sigma]
    dlo_arr = np.ascontiguousarray(
        dlo_sorted.reshape(NCH, 128).T).astype(np.float32)  # [128, NCH]

    rows = [rho[c::NCORES] for c in range(NCORES)]
    adjs = np.asarray(adj)
    in_maps = []
    for c in range(NCORES):
        rc = rows[c]
        Ac = np.ascontiguousarray(adjs[rc][:, sigma].T).astype(fp8_np)
        slo_c = slo_full[rc].astype(np.float32).reshape(1, ROWS)
        sloc_c = np.ascontiguousarray(
            slo_full[rc].reshape(NSLAB, 128).T).astype(np.float32)  # [128, NSLAB]
        in_maps.append({
            "A": Ac,
            "WT": np.ascontiguousarray(Wf.T),
            "hTb": hTbs,
            "hTbown": np.ascontiguousarray(hb.T[:, rc]),
            "W": Wf,
            "a": af,
            "dlo": dlo_arr,
            "slo": slo_c,
            "sloc": sloc_c,
        })
    return kneg, kpos, rows, in_maps


_nc_cache = {}


def get_module(kneg, kpos):
    key = (tuple(kneg), tuple(kpos))
    if key not in _nc_cache:
        lanes = plan_lanes(kneg, kpos)
        _nc_cache[key] = build_module(kneg, kpos, lanes)
    return _nc_cache[key]


def kernel(h, adj, W, a, trace=False, trace_kwargs=None):
    kneg, kpos, rows, in_maps = host_prep(h, adj, W, a)
    nc = get_module(kneg, kpos)
    res = bass_utils.run_bass_kernel_spmd(
        nc, in_maps, core_ids=list(range(NCORES)), trace=trace,
        **(trace_kwargs or {}))
    out = np.empty((N_FULL, F), dtype=np.float32)
    for c in range(NCORES):
        out[rows[c]] = np.asarray(res.results[c]["out"]).astype(np.float32)
    kernel.last_results = res
    return out


if __name__ == "__main__":
    rng = np.random.default_rng(0)
    h = rng.standard_normal((N_FULL, IN_DIM), dtype=np.float32)
    adj = (rng.random((N_FULL, N_FULL)) < 0.5).astype(np.int32)
    W = (rng.standard_normal((IN_DIM, F), dtype=np.float32) / np.sqrt(IN_DIM))
    a = rng.standard_normal(2 * F, dtype=np.float32)
    out = kernel(h, adj, W, a)
    print("out", out.shape, np.abs(out).mean())


# revision 45
# speedup vs baseline: 1.1885x; 1.1269x over previous
"""Trainium2 Bass kernel for nn_EuclideanGATLayer (GAT layer, N=8192) — v5.

Math (per reference):
    Wh = h @ W; s = Wh@a[:F]; d = Wh@a[F:]
    e_ij = leaky_relu(s_i + d_j, 0.01); attn = softmax(mask(e)); out = elu(attn @ Wh)

Identities:
    exp(lrelu(x)) = exp(0.01 s_i) * Q_j * exp(0.99 relu(s_i+d_j)),  Q_j = exp(0.01 d_j)
    exp(0.99 relu(x)) = max(E_i * D_j, 1),  E_i = exp(0.99 s_i), D_j = exp(0.99 d_j)
    (row factor exp(0.01 s_i) cancels in softmax; Q folded into G = [Q*Wh | Q])

Host staging (order-invariant math): columns j sorted by d, rows i sorted by
s and striped across cores. Per (32-row group x 128-col chunk) block:
  NEG  (s_max+d_max < -m): weight exactly Q_j  -> matmul straight off fp8 A
  POS  (s_min+d_min > +m): weight E_i*D_j*Q_j  -> Gpos stationary, E in epilogue
  band (straddles 0):      Z = max(E*D,1)*A elementwise (bf16 lanes)

v5 structure:
  * NEG per chunk-PAIR with one fp8 DoubleRow matmul (stationary fp8(G) pairs
    padded to 80 cols for the 16B pair-stride ISA rule). NEG weights are
    diffuse (Q in [0.73,1.36]) so fp8 stationary costs ~0.1% accuracy.
  * band boundaries at 32-row granularity (global 256-row groups) -> ~4x less
    elementwise band work than 128-row slabs.
  * chunks processed HEAVY-FIRST (descending c): the POS-heavy chunks overlap
    the mask DMA stream; the cheap DoubleRow-NEG chunks form the tail.
  * d/s low bits: host stages exact corrections dlo/sloc/slo vs the device's
    bf16 path (no fp8 h residues; |d_dev - d_fp64| ~ 1e-6).
  * masks arrive in 8 x 1MB DMAs interleaved with hT groups, descending, on
    the SP queue; small setup tensors ride the Pool SWDGE queue (merged into
    two blobs) so they don't serialize ahead of the stream on HWDGE.
"""
import sys

sys.path.insert(0, "/opt/trn_rl_repo")

import numpy as np
from contextlib import ExitStack

import ml_dtypes
import concourse.bass as bass
import concourse.bacc as bacc
import concourse.tile as tile
from concourse import mybir
from concourse import bass_utils
from concourse.masks import make_identity

N_FULL = 8192
IN_DIM = 128
F = 64
NCORES = 8
ROWS = N_FULL // NCORES      # 1024 rows per core
NCH = N_FULL // 128          # 64 j-chunks
NPAIR = NCH // 2
NSLAB = 8                    # epilogue slabs (128 rows each)
SUB = 32                     # band boundary granularity (rows)
MARGIN = 0.02
GP = 80                      # padded fp8-G width (pair stride must be %16)
MB = 8                       # mask chunks per batched DMA
GRP = 8                      # chunks per hT group / G-setup group

f32 = mybir.dt.float32
f32r = mybir.dt.float32r
bf16 = mybir.dt.bfloat16
fp8 = mybir.dt.float8e4
AF = mybir.ActivationFunctionType
OP = mybir.AluOpType
DR = mybir.MatmulPerfMode.DoubleRow

bf16_np = ml_dtypes.bfloat16
fp8_np = ml_dtypes.float8_e4m3


def plan_lanes(rneg, rpos):
    """Band lanes round-robin over DVE / Act+DVE / Pool (Act-heavy: the
    activation engine has the most slack during the stream)."""
    lanes = []
    for c in range(NCH):
        if rneg[c] == rpos[c]:
            lanes.append(None)
        else:
            lanes.append(("LB", "LA", "LP", "LA")[c % 4])
    return lanes


def gat_body(tc, out, A, WTa, blob, slo, hTb, hTbown, rneg, rpos, lanes):
    nc = tc.nc
    ngroups = NCH // GRP

    with ExitStack() as ctx:
        const = ctx.enter_context(tc.tile_pool(name="const", bufs=1))

        # ---- small loads first (the setup chain gates the stream start) ----
        WTasb = const.tile([F, IN_DIM + 2], f32)
        nc.sync.dma_start(out=WTasb, in_=WTa)
        blobsb = const.tile([128, F + NCH + NSLAB], f32)
        nc.sync.dma_start(out=blobsb, in_=blob)
        slosb = const.tile([1, ROWS], f32)
        nc.sync.dma_start(out=slosb, in_=slo)
        hTbownsb = const.tile([128, ROWS], bf16)
        nc.sync.dma_start(out=hTbownsb, in_=hTbown)
        Wsb = blobsb[:, 0:F]
        dlosb = blobsb[:, F:F + NCH]
        slocsb = blobsb[:, F + NCH:F + NCH + NSLAB]

        # ---- stream loads (SP queue / HWDGE), heavy chunks first ----
        hTbg = [const.tile([128, GRP * 128], bf16, name=f"hTbg{g}")
                for g in range(ngroups)]
        Agrp = [const.tile([128, MB, ROWS], fp8, name=f"A{b}")
                for b in range(NCH // MB)]

        def mask_chunk(c):
            return Agrp[c // MB][:, c % MB, :]

        # Interleave so each hT group lands two mask batches before its group's
        # masks: the mask cadence in the heavy phase (3.64us/batch) then
        # matches the heavy pairs' PE cost and the stream never lags.
        def hT_dma(g):
            nc.sync.dma_start(out=hTbg[g],
                              in_=hTb[:, g * GRP * 128:(g + 1) * GRP * 128])

        for g in (ngroups - 1, ngroups - 2, ngroups - 3):
            hT_dma(g)
        for b in range(NCH // MB - 1, -1, -1):
            nc.sync.dma_start(
                out=Agrp[b],
                in_=A[b * MB * 128:(b + 1) * MB * 128, :].rearrange(
                    "(c p) i -> p c i", p=128),
            )
            if b - 3 >= 0:
                hT_dma(b - 3)

        Wa1 = Wa2 = None

        def split_bf16(x_f32, name):
            hi = const.tile([128, 1], bf16, name=f"{name}_hi")
            nc.vector.tensor_copy(hi, x_f32)
            hif = const.tile([128, 1], f32, name=f"{name}_hif")
            nc.vector.tensor_copy(hif, hi)
            res = const.tile([128, 1], f32, name=f"{name}_resf")
            nc.vector.tensor_tensor(out=res, in0=x_f32, in1=hif, op=OP.subtract)
            lo = const.tile([128, 1], bf16, name=f"{name}_lo")
            nc.vector.tensor_copy(lo, res)
            return hi, lo

        W2b = const.tile([128, F + 2], bf16)
        s_stat = const.tile([128, 2], bf16)   # [wb1 | wr1]

        def _do_splits():
            wb1, wr1 = split_bf16(Wa1, "wa1")
            wb2, wr2 = split_bf16(Wa2, "wa2")
            nc.vector.tensor_copy(W2b[:, :F], Wsb)
            nc.vector.tensor_copy(W2b[:, F : F + 1], wb2)
            nc.vector.tensor_copy(W2b[:, F + 1 : F + 2], wr2)
            nc.vector.tensor_copy(s_stat[:, 0:1], wb1)
            nc.vector.tensor_copy(s_stat[:, 1:2], wr1)

        ident = const.tile([128, 128], f32)
        make_identity(nc, ident)
        ones2f = const.tile([2, 128], f32)
        nc.vector.memset(ones2f, 1.0)
        ones2r = const.tile([2, 128], f32r)
        nc.vector.tensor_copy(ones2r, ones2f)
        ones1r = const.tile([1, 128], f32r)
        nc.vector.tensor_copy(ones1r, ones2f[0:1, :])

        s_bcast = const.tile([128, ROWS], f32)
        E_bcast = const.tile([128, ROWS], bf16)
        E_col = const.tile([128, NSLAB], f32)
        Gall = const.tile([128, NCH, F + 1], bf16)
        Gpos = const.tile([128, NCH, F + 1], bf16)
        G8 = const.tile([128, NCH, GP], fp8)
        nc.gpsimd.memset(G8[:, :, F + 1:GP], 0.0)
        znull = const.tile([128, 512], bf16)
        nc.vector.memset(znull, 0.0)
        dall = const.tile([128, NCH], f32)
        d99 = const.tile([128, NCH], f32)
        Dall = const.tile([128, NCH], f32)
        Qall = const.tile([128, NCH], f32)

        otpool = ctx.enter_context(tc.tile_pool(name="ot_ps", bufs=1, space="PSUM"))
        ot = [otpool.tile([GP, 512], f32, tag=f"ot{t}", name=f"ot{t}") for t in range(2)]
        otp = [otpool.tile([F + 1, 512], f32, tag=f"otp{t}", name=f"otp{t}")
               for t in range(2)]

        # zero-init all four accumulators via dummy matmuls (also warms PE)
        for h in range(2):
            nc.tensor.matmul(ot[h], znull[:, :GP], znull, start=True, stop=False,
                             skip_group_check=True)
            nc.tensor.matmul(otp[h], znull[:, :F + 1], znull, start=True, stop=False,
                             skip_group_check=True)

        with ExitStack() as sctx:
            spool = sctx.enter_context(tc.tile_pool(name="setup_s_ps", bufs=2, space="PSUM"))
            wa_ps = spool.tile([128, 2], f32, tag="wa", bufs=1)
            nc.tensor.matmul(wa_ps, WTasb[:, :IN_DIM], WTasb[:, IN_DIM:IN_DIM + 2],
                             start=True, stop=True)
            Wa1 = const.tile([128, 1], f32)
            nc.vector.tensor_copy(Wa1, wa_ps[:, 0:1])
            Wa2 = const.tile([128, 1], f32)
            nc.vector.tensor_copy(Wa2, wa_ps[:, 1:2])
            _do_splits()
            # s rows: 2 bf16 pieces + the exact lo correction row
            for j0 in range(0, ROWS, 512):
                s_ps_a = spool.tile([2, 512], f32, tag="sa", bufs=1)
                nc.tensor.matmul(s_ps_a, s_stat, hTbownsb[:, j0:j0 + 512],
                                 start=True, stop=True)
                s_sb_a = const.tile([2, 512], f32r, name=f"ssba{j0}")
                nc.vector.tensor_copy(s_sb_a, s_ps_a)
                s_sb_b = const.tile([1, 512], f32r, name=f"ssbb{j0}")
                nc.vector.tensor_copy(s_sb_b, slosb[:, j0:j0 + 512])
                sb_ps = spool.tile([128, 512], f32, tag="sb", bufs=1)
                nc.tensor.matmul(sb_ps, ones2r, s_sb_a, start=True, stop=False)
                nc.tensor.matmul(sb_ps, ones1r, s_sb_b, start=False, stop=True)
                nc.vector.tensor_copy(s_bcast[:, j0:j0 + 512], sb_ps)
            nc.scalar.activation(E_bcast, s_bcast, AF.Exp, scale=0.99)

        # ---- main stream (Wh/G setup for group g woven in before its pairs) ----
        whpool = ctx.enter_context(tc.tile_pool(name="setup_wh_ps", bufs=2, space="PSUM"))
        zpool = ctx.enter_context(tc.tile_pool(name="zpool", bufs=28))
        tpool = ctx.enter_context(tc.tile_pool(name="tpool", bufs=20))
        upool = ctx.enter_context(tc.tile_pool(name="upool", bufs=12))

        def setup_scol():
            wh_s = whpool.tile([128, GRP, 128], f32, tag="wh")
            for b in range(NSLAB):
                own = hTbownsb[:, b * 128:(b + 1) * 128]
                nc.tensor.matmul(wh_s[:, b, 0:2], own, s_stat,
                                 start=True, stop=True)
            scsum = const.tile([128, NSLAB], f32, name="scsum")
            nc.vector.tensor_reduce(out=scsum, in_=wh_s[:, :, 0:2],
                                    axis=mybir.AxisListType.X, op=OP.add)
            scs2 = const.tile([128, NSLAB], f32, name="scs2")
            nc.vector.tensor_tensor(out=scs2, in0=scsum, in1=slocsb, op=OP.add)
            nc.scalar.activation(E_col, scs2, AF.Exp, scale=0.99)

        def setup_group(g):
            wh_grp = whpool.tile([128, GRP, 128], f32, tag="wh")
            for cc in range(GRP):
                j0 = cc * 128
                nc.tensor.matmul(wh_grp[:, cc, :F + 2], hTbg[g][:, j0:j0 + 128],
                                 W2b[:, :F + 2], start=True, stop=True)
            sl = slice(g * GRP, (g + 1) * GRP)
            dhi = const.tile([128, GRP], f32, name=f"dhi{g}")
            nc.vector.tensor_reduce(out=dhi, in_=wh_grp[:, :, F:F + 2],
                                    axis=mybir.AxisListType.X, op=OP.add)
            nc.vector.tensor_tensor(out=dall[:, sl], in0=dhi, in1=dlosb[:, sl],
                                    op=OP.add)
            nc.vector.tensor_scalar(out=d99[:, sl], in0=dall[:, sl], scalar1=0.99,
                                    scalar2=None, op0=OP.mult)
            nc.scalar.activation(Dall[:, sl], d99[:, sl], AF.Exp)
            nc.scalar.activation(Qall[:, sl], dall[:, sl], AF.Exp, scale=0.01)
            qb = bass.AP(tensor=Qall.tensor, offset=Qall.offset + g * GRP,
                         ap=[Qall.ap[0], [1, GRP], [0, F]])
            nc.vector.tensor_tensor(out=Gall[:, sl, :F], in0=wh_grp[:, :, :F],
                                    in1=qb, op=OP.mult)
            nc.vector.tensor_copy(Gall[:, sl, F], Qall[:, sl])
            # Gpos = D (bcast) * Gall: num cols D*Q*Wh, den col D*Q = exp(d)
            db_ = bass.AP(tensor=Dall.tensor, offset=Dall.offset + g * GRP,
                          ap=[Dall.ap[0], [1, GRP], [0, F + 1]])
            geng = nc.vector if g % 2 == 0 else nc.gpsimd
            geng.tensor_tensor(out=Gpos[:, sl, :], in0=Gall[:, sl, :],
                               in1=db_, op=OP.mult)
            # fp8 copy of G for the DoubleRow NEG matmuls (diffuse weights)
            nc.scalar.activation(G8[:, sl, 0:F + 1], Gall[:, sl, :], AF.Copy)

        setup_scol()
        setup_group(ngroups - 1)
        setup_group(ngroups - 2)

        small = ctx.enter_context(tc.tile_pool(name="small", bufs=2))
        otpsb = {}
        hpe = None
        # last processed pair that still writes otp (smallest t with POS)
        t_pos_min = min((t for t in range(NPAIR)
                         if rpos[2 * t] < ROWS or rpos[2 * t + 1] < ROWS),
                        default=NPAIR - 1)
        idx_pos_done = NPAIR - 1 - t_pos_min

        for idx in range(NPAIR):
            t = NPAIR - 1 - idx
            c0, c1 = 2 * t, 2 * t + 1
            if idx % 4 == 0 and idx // 4 <= 5:
                setup_group(5 - idx // 4)

            rz = min(rneg[c0], rneg[c1])
            # ---- paired NEG via fp8 DoubleRow: weight Q_j, moving A ----
            if rz > 0:
                b_t = c0 // MB
                sub = c0 % MB
                for h in range(2):
                    lo, hi = 512 * h, 512 * (h + 1)
                    r = min(rz, hi)
                    if r > lo:
                        nc.tensor.matmul(
                            ot[h][:, 0:r - lo],
                            G8[:, c0:c0 + 2, :],
                            Agrp[b_t][:, sub:sub + 2, lo:r],
                            start=False, stop=False, perf_mode=DR,
                            skip_group_check=True)

            for c in (c0, c1):
                r1 = rpos[c]
                # Z covers [rz, r1): the own-NEG sliver has max(ED,1)==1 so
                # Z == A there exactly; one band matmul replaces sliver+band.
                # When there is no band (rneg==rpos) the region is pure NEG
                # and the moving operand is A itself.
                r0 = rz
                lane = lanes[c]
                Ac = mask_chunk(c)
                Z = None
                w = r1 - r0
                assert w <= 256, f"band+sliver width {w} exceeds lane tiles"
                if lane is not None and w > 0:
                    sl_b = slice(r0, r1)
                    if lane == "LB":
                        t_t = tpool.tile([128, 256], bf16, tag="t")
                        nc.vector.tensor_scalar(out=t_t[:, :w], in0=E_bcast[:, sl_b],
                                                scalar1=Dall[:, c:c + 1], scalar2=1.0,
                                                op0=OP.mult, op1=OP.max)
                        Z = zpool.tile([128, 256], bf16, tag="z")
                        nc.vector.tensor_tensor(out=Z[:, :w], in0=t_t[:, :w],
                                                in1=Ac[:, sl_b], op=OP.mult)
                    elif lane == "LP":
                        t_t = tpool.tile([128, 256], bf16, tag="t")
                        nc.gpsimd.tensor_scalar(out=t_t[:, :w], in0=E_bcast[:, sl_b],
                                                scalar1=Dall[:, c:c + 1], scalar2=1.0,
                                                op0=OP.mult, op1=OP.max)
                        Z = zpool.tile([128, 256], bf16, tag="z")
                        nc.gpsimd.tensor_tensor(out=Z[:, :w], in0=t_t[:, :w],
                                                in1=Ac[:, sl_b], op=OP.mult)
                    else:  # LA
                        u = upool.tile([128, 256], bf16, tag="u")
                        nc.scalar.activation(u[:, :w], s_bcast[:, sl_b], AF.Exp,
                                             bias=d99[:, c:c + 1], scale=0.99)
                        Z = zpool.tile([128, 256], bf16, tag="z")
                        nc.vector.scalar_tensor_tensor(out=Z[:, :w], in0=u[:, :w],
                                                       scalar=1.0, in1=Ac[:, sl_b],
                                                       op0=OP.max, op1=OP.mult)

                G_c = Gall[:, c, :]
                Gp_c = Gpos[:, c, :]
                for half in range(2):
                    lo, hi = 512 * half, 512 * (half + 1)
                    b0 = min(max(r0, lo), hi)   # NEG | band boundary
                    b1 = min(max(r1, lo), hi)   # band | POS boundary
                    if b1 > b0:  # band+sliver: moving = Z (or A when no band)
                        mv = Z[:, b0 - r0:b1 - r0] if Z is not None else Ac[:, b0:b1]
                        nc.tensor.matmul(ot[half][:F + 1, b0 - lo:b1 - lo],
                                         G_c, mv, start=False, stop=False,
                                         skip_group_check=True)
                    if hi > b1:  # POS: weight E_i*D_j*Q_j -> moving = A vs Gpos
                        nc.tensor.matmul(otp[half][:, b1 - lo:],
                                         Gp_c, Ac[:, b1:hi], start=False, stop=False,
                                         skip_group_check=True)
            if idx == idx_pos_done:
                # otp saw its last POS matmul: close it and run its side of
                # the epilogue (copies, transposes, E_col multiply) overlapped
                # with the remaining cheap NEG pairs.
                for h in range(2):
                    nc.tensor.matmul(otp[h][:, :16], Gall[:, 0, :], znull[:, :16],
                                     start=False, stop=True, skip_group_check=True)
                otpsb[0] = small.tile([F + 1, 512], f32, tag="otpsb", name="otpsb0")
                nc.scalar.activation(otpsb[0], otp[0], AF.Copy)
                otpsb[1] = small.tile([F + 1, 512], f32, tag="otpsb", name="otpsb1")
                nc.vector.tensor_copy(otpsb[1], otp[1])
                tpP = whpool.tile([128, GRP, 128], f32, tag="wh")
                for half in range(2):
                    for q in range(4):
                        nc.tensor.transpose(tpP[:, 4 * half + q, :F + 1],
                                            otpsb[half][:, q * 128:(q + 1) * 128],
                                            ident[:F + 1, :F + 1])
                ecb = bass.AP(tensor=E_col.tensor, offset=E_col.offset,
                              ap=[E_col.ap[0], [1, GRP], [0, F + 1]])
                hpe = small.tile([128, GRP, F + 1], f32, tag="hpe")
                nc.vector.tensor_tensor(out=hpe, in0=tpP[:, :, :F + 1], in1=ecb,
                                        op=OP.mult)
            if idx == NPAIR - 1:
                for h in range(2):
                    nc.tensor.matmul(ot[h][:F + 1, :16], Gall[:, 0, :], znull[:, :16],
                                     start=False, stop=True, skip_group_check=True)

        # ---- epilogue (ot side + per-quarter pipelined math) ----
        otsb = {}
        otsb[0] = small.tile([F + 1, 512], f32, tag="otsb", name="otsb0")
        nc.vector.tensor_copy(otsb[0], ot[0][:F + 1, :])
        otsb[1] = small.tile([F + 1, 512], f32, tag="otsb", name="otsb1")
        nc.scalar.activation(otsb[1], ot[1][:F + 1, :], AF.Copy)
        tpO = whpool.tile([128, GRP, 128], f32, tag="wh")
        hptot = small.tile([128, GRP, F + 1], f32, tag="hptot")
        dens = small.tile([128, GRP], f32, tag="dens")
        hpre = small.tile([128, GRP, F], f32, tag="hpre")
        emin = small.tile([128, GRP, F], f32, tag="emin")
        eexp = small.tile([128, GRP, F], f32, tag="eexp")
        relu1 = small.tile([128, GRP, F], f32, tag="relu1")
        otf = small.tile([128, GRP, F], bf16, tag="otf")
        for half in range(2):
            for q in range(4):
                nc.tensor.transpose(tpO[:, 4 * half + q, :F + 1],
                                    otsb[half][:, q * 128:(q + 1) * 128],
                                    ident[:F + 1, :F + 1])
        for qq in range(4):   # quarters = slab pairs, pipelined across engines
            sq = slice(2 * qq, 2 * qq + 2)
            eng = (nc.vector, nc.gpsimd)[qq % 2]
            oth = (nc.gpsimd, nc.vector)[qq % 2]
            oth.tensor_tensor(out=hptot[:, sq, :], in0=tpO[:, sq, :F + 1],
                              in1=hpe[:, sq, :], op=OP.add)
            nc.vector.reciprocal(dens[:, sq], hptot[:, sq, F])
            db = bass.AP(tensor=dens.tensor, offset=dens.offset + 2 * qq,
                         ap=[dens.ap[0], [1, 2], [0, F]])
            eng.tensor_tensor(out=hpre[:, sq, :], in0=hptot[:, sq, :F], in1=db,
                              op=OP.mult)
            # elu(x) = relu(x) - 1 + exp(min(x, 0))
            eng.tensor_scalar(out=emin[:, sq, :], in0=hpre[:, sq, :], scalar1=0.0,
                              scalar2=None, op0=OP.min)
            nc.scalar.activation(eexp[:, sq, :], emin[:, sq, :], AF.Exp)
            oth.tensor_scalar(out=relu1[:, sq, :], in0=hpre[:, sq, :], scalar1=0.0,
                              scalar2=-1.0, op0=OP.max, op1=OP.add)
            eng.tensor_tensor(out=otf[:, sq, :], in0=relu1[:, sq, :],
                              in1=eexp[:, sq, :], op=OP.add)
            nc.sync.dma_start(
                out=out[256 * qq:256 * (qq + 1), :].rearrange(
                    "(b p) f -> p b f", p=128),
                in_=otf[:, sq, :])


def build_module(rneg, rpos, lanes):
    nc = bacc.Bacc("TRN2", target_bir_lowering=False, debug=False,
                   enable_asserts=True, num_devices=NCORES)
    A = nc.dram_tensor("A", [N_FULL, ROWS], fp8, kind="ExternalInput").ap()
    WTa = nc.dram_tensor("WTa", [F, IN_DIM + 2], f32, kind="ExternalInput").ap()
    blob = nc.dram_tensor("blob", [128, F + NCH + NSLAB], f32,
                          kind="ExternalInput").ap()
    slo = nc.dram_tensor("slo", [1, ROWS], f32, kind="ExternalInput").ap()
    hTb = nc.dram_tensor("hTb", [IN_DIM, N_FULL], bf16, kind="ExternalInput").ap()
    hTbown = nc.dram_tensor("hTbown", [IN_DIM, ROWS], bf16, kind="ExternalInput").ap()
    out = nc.dram_tensor("out", [ROWS, F], bf16, kind="ExternalOutput").ap()
    with tile.TileContext(nc) as tc:
        gat_body(tc, out, A, WTa, blob, slo, hTb, hTbown, rneg, rpos, lanes)
    nc.compile()
    return nc


def host_prep(h, adj, W, a):
    h64 = np.asarray(h, dtype=np.float64)
    W64 = np.asarray(W, dtype=np.float64)
    a64 = np.asarray(a, dtype=np.float64)
    Wh = h64 @ W64
    s_full = Wh @ a64[:F]
    d_full = Wh @ a64[F:]
    sigma = np.argsort(d_full, kind="stable")
    rho = np.argsort(s_full, kind="stable")
    s_sorted = s_full[rho]
    d_sorted = d_full[sigma]

    # 32-row (per-core) = 256-row (global) band boundaries per chunk
    NG = ROWS // SUB   # 32 groups
    gmax = s_sorted.reshape(NG, SUB * NCORES).max(axis=1)
    gmin = s_sorted.reshape(NG, SUB * NCORES).min(axis=1)
    rneg, rpos = [], []
    for c in range(NCH):
        dmax = d_sorted[128 * (c + 1) - 1]
        dmin = d_sorted[128 * c]
        k = 0
        while k < NG and gmax[k] + dmax < -MARGIN:
            k += 1
        rneg.append(SUB * k)
        p = NG
        while p > k and gmin[p - 1] + dmin > MARGIN:
            p -= 1
        rpos.append(SUB * p)

    # Fold small POS regions into the Z band where [min-pair-rneg, ROWS) fits
    # the 256-wide lane tiles: Z = max(E*D,1)*A is exact there (bf16 holds
    # e^~23), removes 2 tiny POS matmuls per chunk in the cheap tail pairs,
    # and lets otp close (and its epilogue start) several pairs early.
    for t in range(NPAIR):
        c0, c1 = 2 * t, 2 * t + 1
        rz = min(rneg[c0], rneg[c1])
        if ROWS - rz <= 256:
            rpos[c0] = ROWS
            rpos[c1] = ROWS

    hf = np.asarray(h, dtype=np.float32)
    hb = hf.astype(bf16_np)
    hb64 = hb.astype(np.float64)
    hTbs = np.ascontiguousarray(hb.T[:, sigma])

    # replicate the device's Wa splits to compute exact lo-corrections
    Wf = np.asarray(W, dtype=np.float32)
    af = np.asarray(a, dtype=np.float32)
    Wa12 = W64 @ a64.reshape(2, F).T    # [:, 0] = W@a1, [:, 1] = W@a2
    Wa1f = Wa12[:, 0].astype(np.float32)
    Wa2f = Wa12[:, 1].astype(np.float32)

    def splits(x):
        hi = x.astype(bf16_np)
        lo = (x - hi.astype(np.float32)).astype(bf16_np)
        return hi.astype(np.float64) + lo.astype(np.float64)

    w1 = splits(Wa1f)   # wb1 + wr1 as the device sees them
    w2 = splits(Wa2f)
    d_hi = hb64 @ w2
    s_hi = hb64 @ w1
    dlo_full = d_full - d_hi
    slo_full = s_full - s_hi

    dlo_arr = np.ascontiguousarray(
        dlo_full[sigma].reshape(NCH, 128).T).astype(np.float32)  # [128, NCH]
    WTa_arr = np.concatenate(
        [np.ascontiguousarray(Wf.T),
         af.reshape(2, F).T.astype(np.float32)], axis=1)          # [F, 130]

    rows = [rho[c::NCORES] for c in range(NCORES)]
    adjs = np.asarray(adj)
    in_maps = []
    for c in range(NCORES):
        rc = rows[c]
        Ac = np.ascontiguousarray(adjs[rc][:, sigma].T).astype(fp8_np)
        slo_c = slo_full[rc].astype(np.float32).reshape(1, ROWS)
        sloc_c = np.ascontiguousarray(
            slo_full[rc].reshape(NSLAB, 128).T).astype(np.float32)  # [128, NSLAB]
        blob_c = np.concatenate([Wf, dlo_arr, sloc_c], axis=1)      # [128, 136]
        in_maps.append({
            "A": Ac,
            "WTa": WTa_arr,
            "blob": np.ascontiguousarray(blob_c),
            "slo": slo_c,
            "hTb": hTbs,
            "hTbown": np.ascontiguousarray(hb.T[:, rc]),
        })
    return rneg, rpos, rows, in_maps


_nc_cache = {}


def get_module(rneg, rpos):
    key = (tuple(rneg), tuple(rpos))
    if key not in _nc_cache:
        lanes = plan_lanes(rneg, rpos)
        _nc_cache[key] = build_module(rneg, rpos, lanes)
    return _nc_cache[key]


def kernel(h, adj, W, a, trace=False, trace_kwargs=None):
    rneg, rpos, rows, in_maps = host_prep(h, adj, W, a)
    nc = get_module(rneg, rpos)
    res = bass_utils.run_bass_kernel_spmd(
        nc, in_maps, core_ids=list(range(NCORES)), trace=trace,
        **(trace_kwargs or {}))
    out = np.empty((N_FULL, F), dtype=np.float32)
    for c in range(NCORES):
        out[rows[c]] = np.asarray(res.results[c]["out"]).astype(np.float32)
    kernel.last_results = res
    return out


if __name__ == "__main__":
    rng = np.random.default_rng(0)
    h = rng.standard_normal((N_FULL, IN_DIM), dtype=np.float32)
    adj = (rng.random((N_FULL, N_FULL)) < 0.5).astype(np.int32)
    W = (rng.standard_normal((IN_DIM, F), dtype=np.float32) / np.sqrt(IN_DIM))
    a = rng.standard_normal(2 * F, dtype=np.float32)
    out = kernel(h, adj, W, a)
    print("out", out.shape, np.abs(out).mean())
